# revision 42
# baseline (speedup 1.0000x reference)
"""Trainium2 Bass kernel for nn_AbstractSNClustering (moe_routing).

Reference computation (B=1048576, DX=32, DS=16, H=128, K=64, NSN=4):
    x_tune   = sigmoid(hidden @ W_tune + b_tune)                  [B,1]
    dist     = ||c_k||^2 - 2 x.c_k ; cl = argmin_k dist           [B]
    x_sn     = einsum(W_sn[:,cl,:], s) + b_sn[:,cl].T             [B,NSN]
    x_sn_sum = sum(x_sn * rsw[cl], -1)                            [B,1]
    out      = x_sn_sum + x_tune * (naive_pred - x_sn_sum)

Key algebraic collapse: both the mixture weights and subnet params are
indexed by the same cluster id, so
    x_sn_sum[b] = s[b].WW[cl] + BB[cl],
    WW[k,d] = sum_n rsw[k,n] W_sn[n,k,d],  BB[k] = sum_n rsw[k,n] b_sn[n,k]
(host-precomputed).  On device, one matmul per 128-row tile against a
constant [50,128] table computes both dist[b,k] and t[b,k]=s'.WW[k]+BB[k].
Routing is DVE-only: reduce_min, one tensor_tensor is_equal with
broadcast row-min (one-hot), mult + reduce_add (select t at argmin).
Tune head z = hidden.W_tune via fused scalar_tensor_tensor accumulate.

Performance structure (measured on HW via For_i-slope timing):
  - inputs x,s (+ the two ones columns folding in ||c||^2 and BB) are
    pre-concatenated host-side into one [B,50] tensor -> one contiguous
    DMA per 4096-row macro-chunk (strided SBUF writes were 3x slow).
  - 4096-row DMA macro-chunks (5 dma_starts per macro; many small DMAs
    cost ~0.9us each), hidden on the scalar-HWDGE ring, rest on sync.
  - compute in 1024-row sub-chunks: PE transposes [128,50]->[50,128],
    per-128-row matmul vs the table, DVE routing, ACT sigmoid.
  ~648us/exec vs ~291us DMA-only floor (theoretical ~253us at 358GB/s).

Sharding: pure data parallel over B across 8 NeuronCores; parameter
tables replicated.
"""

import contextlib
import os
import sys

sys.path.insert(0, "/opt/trn_rl_repo")

import numpy as np

import concourse.bass as bass
import concourse.mybir as mybir
from concourse import bacc, tile
from concourse.bass_utils import run_bass_kernel_spmd

B, DX, DS, H, K, NSN = 1048576, 32, 16, 128, 64, 4
NCORES = 8
ROWS = B // NCORES          # rows per core
CHUNK = int(os.environ.get("K_CHUNK", "1024"))  # rows per compute sub-chunk
CPT = CHUNK // 128          # sub-rows per partition per sub-chunk
MCH = int(os.environ.get("K_MCH", "4096"))     # rows per DMA macro-chunk
NF = DX + 1 + DS + 1        # 50 contraction features: [x | 1 | s | 1]
XCOL1 = DX                  # ones col for x (c2 row)
SCOL0 = DX + 1              # s cols start
SCOL1 = DX + 1 + DS         # ones col for s (BB row)

f32 = mybir.dt.float32
bf16 = mybir.dt.bfloat16
Alu = mybir.AluOpType
Act = mybir.ActivationFunctionType
AX = mybir.AxisListType

# tuning knobs (env-overridable for A/B benching)
ISEQ_MODE = os.environ.get("K_ISEQ", "tt_bcast")   # "ts" | "tt_bcast"
Z_MODE = os.environ.get("K_Z", "stt")         # "tt" | "stt"
HID_BF16 = os.environ.get("K_HBF16", "0") == "1"
XSN_MODE = os.environ.get("K_XSN", "tt")     # "tt" | "stt"
BLEND_GPSIMD = os.environ.get("K_BLGP", "0") == "1"  # blend ops on GPSIMD
IO_BUFS = int(os.environ.get("K_IOB", "2"))
MID_BUFS = int(os.environ.get("K_MIDB", "3"))
DMASK = int(os.environ.get("K_DMASK", "15"))  # stage0 DMA attribution mask
USE_TTR = False     # fused tensor_tensor_reduce (crashes device on this runtime)


def _build(
    rows: int, b_tune_val: float, stage: int = 99, reps: int = 1
) -> bass.Bass:
    """stage: 0=DMA only, 99=full
    reps: on-device repeat count (timing only; >1 wraps the kernel in For_i)"""
    mch = min(MCH, rows)            # rows per DMA macro-chunk
    cpm = mch // 128                # rows per partition per macro
    nsub = mch // CHUNK             # compute sub-chunks per macro
    nmacro = rows // mch
    assert rows % mch == 0 and mch % CHUNK == 0
    nc = bacc.Bacc(None)

    hdt = bf16 if HID_BF16 else f32
    xs_ext = nc.declare_dram_parameter("xs1", [rows, NF], f32, isOutput=False)
    h_ext = nc.declare_dram_parameter("hidden", [rows, H], f32, isOutput=False)
    nv_ext = nc.declare_dram_parameter("naive", [rows, 1], f32, isOutput=False)
    tbl_ext = nc.declare_dram_parameter("bigtable", [NF, 128], f32, isOutput=False)
    idn_ext = nc.declare_dram_parameter("ident", [128, 128], f32, isOutput=False)
    wtr_ext = nc.declare_dram_parameter("wtune_rep", [128, H], hdt, isOutput=False)
    out_ext = nc.declare_dram_parameter("out", [rows, 1], f32, isOutput=True)

    with tile.TileContext(nc) as tc:
        with (
            tc.tile_pool(name="consts", bufs=1) as cpool,
            tc.tile_pool(name="io", bufs=IO_BUFS) as io,
            tc.tile_pool(name="mid", bufs=MID_BUFS) as mid,
            tc.tile_pool(name="pst", bufs=2, space="PSUM") as pst,
            tc.tile_pool(name="psm", bufs=2, space="PSUM") as psm,
        ):
            ident = cpool.tile([128, 128], f32, tag="ident")
            nc.sync.dma_start(ident[:], idn_ext[:])
            btbl = cpool.tile([NF, 128], f32, tag="btbl")
            nc.sync.dma_start(btbl[:], tbl_ext[:])
            wtr = cpool.tile([128, H], hdt, tag="wtr")
            nc.sync.dma_start(wtr[:], wtr_ext[:])
            btn = cpool.tile([128, 1], f32, tag="btn")
            nc.vector.memset(btn[:], b_tune_val)

            loop_cm = tc.For_i(0, reps, 1) if reps > 1 else contextlib.nullcontext()
            with loop_cm:
              for m in range(nmacro):
                m0 = m * mch
                ct = io.tile([128, cpm * NF], f32, tag="catm")
                c3m = ct[:].rearrange("p (c f) -> p c f", c=cpm)

                # macro DMAs: row b = m0 + p*cpm + c  (per-partition contiguous)
                if stage >= 1 or DMASK & 1:
                    nc.sync.dma_start(
                        ct[:],
                        xs_ext[m0 : m0 + mch, :].rearrange(
                            "(p c) f -> p (c f)", c=cpm
                        ),
                    )
                else:
                    nc.vector.memset(ct[:, 0:1], 0.0)
                htm = io.tile([128, cpm * H], hdt, tag="htm")
                h_src = h_ext[m0 : m0 + mch, :].rearrange("(p c) d -> p (c d)", c=cpm)
                if stage >= 1 or DMASK & 4:
                    if HID_BF16:
                        nc.gpsimd.dma_start(htm[:], h_src)   # SWDGE dtype-cast DMA
                    else:
                        nc.scalar.dma_start(htm[:], h_src)   # second HWDGE ring
                elif stage < 1:
                    nc.vector.memset(htm[:, 0:1], 0.0)
                h3m = htm[:].rearrange("p (c f) -> p c f", c=cpm)
                nvm = io.tile([128, cpm], f32, tag="nvm")
                nc.sync.dma_start(
                    nvm[:],
                    nv_ext[m0 : m0 + mch, :].rearrange("(p c) o -> p (c o)", c=cpm),
                )
                resm = mid.tile([128, cpm], f32, tag="resm")

                if stage < 1:
                    nc.vector.tensor_copy(resm[:], nvm[:])
                    nc.vector.tensor_add(resm[:, 0:1], resm[:, 0:1], htm[:, 0:1])
                    nc.vector.tensor_add(resm[:, 0:1], resm[:, 0:1], ct[:, 0:1])
                else:
                  for sc in range(nsub):
                    c0 = sc * CPT
                    # [x|1|s|1] -> transposed [50, 128] per 128-row sub-tile
                    xsT_ps = pst.tile([NF, CPT * 128], f32, tag="xsT_ps")
                    for c in range(CPT):
                        nc.tensor.transpose(
                            xsT_ps[:, c * 128 : (c + 1) * 128],
                            c3m[:, c0 + c, :],
                            ident[:],
                        )
                    xsT = mid.tile([NF, CPT * 128], f32, tag="xsT")
                    nc.scalar.copy(xsT[:], xsT_ps[:])

                    # one matmul per sub-tile: out cols = [dist(64) | t(64)]
                    dt_ps = psm.tile([128, CPT * 128], f32, tag="dt_ps")
                    d3 = dt_ps[:].rearrange("p (c f) -> p c f", c=CPT)
                    for c in range(CPT):
                        nc.tensor.matmul(
                            d3[:, c, :],
                            xsT[:, c * 128 : (c + 1) * 128],
                            btbl[:],
                            start=True,
                            stop=True,
                        )

                    # routing: row-min over k, one-hot, select t at argmin
                    rmin = mid.tile([128, CPT], f32, tag="rmin")
                    nc.vector.tensor_reduce(
                        rmin[:], d3[:, :, 0:K], axis=AX.X, op=Alu.min
                    )
                    xsn = mid.tile([128, CPT], f32, tag="xsn")
                    scr = mid.tile([128, CPT * K], f32, tag="scr")
                    sc3 = scr[:].rearrange("p (c f) -> p c f", c=CPT)
                    if XSN_MODE == "stt2":
                        # fused routing: ACT copies t to SBUF, then one DVE
                        # scalar_tensor_tensor per sub-tile computes
                        # (dist == rmin) * t with accumulate -> xsn
                        tsb = mid.tile([128, CPT * K], f32, tag="tsb")
                        ts3 = tsb[:].rearrange("p (c f) -> p c f", c=CPT)
                        nc.scalar.copy(ts3[:, :, :], d3[:, :, K : 2 * K])
                        for c in range(CPT):
                            nc.vector.scalar_tensor_tensor(
                                out=sc3[:, c, :],
                                in0=d3[:, c, 0:K],
                                scalar=rmin[:, c : c + 1],
                                in1=ts3[:, c, :],
                                op0=Alu.is_equal,
                                op1=Alu.mult,
                                accum_out=xsn[:, c : c + 1],
                            )
                    if XSN_MODE != "stt2":
                        oh = mid.tile([128, CPT * K], f32, tag="oh")
                        oh3 = oh[:].rearrange("p (c f) -> p c f", c=CPT)
                    if XSN_MODE == "stt2":
                        pass
                    elif ISEQ_MODE == "tt_bcast":
                        rb = rmin[:].unsqueeze(2).broadcast_to([128, CPT, K])
                        nc.vector.tensor_tensor(
                            oh3[:, :, :], d3[:, :, 0:K], rb, op=Alu.is_equal
                        )
                    else:
                        for c in range(CPT):
                            nc.vector.tensor_scalar(
                                oh3[:, c, :], d3[:, c, 0:K],
                                rmin[:, c : c + 1], None, op0=Alu.is_equal,
                            )
                    if XSN_MODE == "stt2":
                        pass
                    elif XSN_MODE == "stt":
                        for c in range(CPT):
                            nc.vector.scalar_tensor_tensor(
                                out=sc3[:, c, :],
                                in0=d3[:, c, K : 2 * K],
                                scalar=1.0,
                                in1=oh3[:, c, :],
                                op0=Alu.bypass,
                                op1=Alu.mult,
                                accum_out=xsn[:, c : c + 1],
                            )
                    else:
                        nc.vector.tensor_mul(
                            sc3[:, :, :], d3[:, :, K : 2 * K], oh3[:, :, :]
                        )
                        nc.vector.tensor_reduce(
                            xsn[:], sc3[:, :, :], axis=AX.X, op=Alu.add
                        )

                    # tune head: z = hidden @ W_tune, sigmoid(z + b_tune)
                    z = mid.tile([128, CPT], f32, tag="z")
                    scz = mid.tile([128, CPT * H], hdt, tag="scz")
                    z3 = scz[:].rearrange("p (c f) -> p c f", c=CPT)
                    if Z_MODE == "stt":
                        for c in range(CPT):
                            nc.vector.scalar_tensor_tensor(
                                out=z3[:, c, :],
                                in0=h3m[:, c0 + c, :],
                                scalar=1.0,
                                in1=wtr[:],
                                op0=Alu.bypass,
                                op1=Alu.mult,
                                accum_out=z[:, c : c + 1],
                            )
                    elif Z_MODE == "gpstt":
                        # fully fused on GPSIMD: mult + accumulate
                        for c in range(CPT):
                            nc.gpsimd.scalar_tensor_tensor(
                                out=z3[:, c, :],
                                in0=h3m[:, c0 + c, :],
                                scalar=1.0,
                                in1=wtr[:],
                                op0=Alu.bypass,
                                op1=Alu.mult,
                                accum_out=z[:, c : c + 1],
                            )
                    elif Z_MODE == "gp":
                        # multiply on GPSIMD (otherwise idle), reduce on DVE
                        wb = wtr[:].unsqueeze(1).broadcast_to([128, CPT, H])
                        nc.gpsimd.tensor_mul(
                            z3[:, :, :], h3m[:, c0 : c0 + CPT, :], wb
                        )
                        nc.vector.tensor_reduce(
                            z[:], z3[:, :, :], axis=AX.X, op=Alu.add
                        )
                    else:
                        for c in range(CPT):
                            nc.vector.tensor_mul(
                                z3[:, c, :], h3m[:, c0 + c, :], wtr[:]
                            )
                        nc.vector.tensor_reduce(
                            z[:], z3[:, :, :], axis=AX.X, op=Alu.add
                        )
                    sig = mid.tile([128, CPT], f32, tag="sig")
                    nc.scalar.activation(sig[:], z[:], Act.Sigmoid, bias=btn[:, 0:1])

                    # blend: out = xsn + sig * (naive - xsn)
                    eng = nc.gpsimd if BLEND_GPSIMD else nc.vector
                    dd = mid.tile([128, CPT], f32, tag="dd")
                    eng.tensor_sub(dd[:], nvm[:, c0 : c0 + CPT], xsn[:])
                    mm = mid.tile([128, CPT], f32, tag="mm")
                    eng.tensor_mul(mm[:], dd[:], sig[:])
                    eng.tensor_add(resm[:, c0 : c0 + CPT], xsn[:], mm[:])

                nc.scalar.dma_start(
                    out_ext[m0 : m0 + mch, :].rearrange("(p c) o -> p (c o)", c=cpm),
                    resm[:],
                )
    if not nc.is_finalized():
        nc.finalize()
    return nc


def _prep_tables(centers, W_tune, b_tune, W_sn, b_sn, running_sn_weight):
    centers = np.asarray(centers, np.float32)
    W_sn = np.asarray(W_sn, np.float32)
    b_sn = np.asarray(b_sn, np.float32)
    rsw = np.asarray(running_sn_weight, np.float32)
    c2 = (centers * centers).sum(1)                      # [K]
    WW = np.einsum("kn,nkd->kd", rsw, W_sn)              # [K, DS]
    BB = np.einsum("kn,nk->k", rsw, b_sn)                # [K]
    tbl = np.zeros((NF, 128), np.float32)
    tbl[0:DX, 0:K] = -2.0 * centers.T                    # dist linear term
    tbl[XCOL1, 0:K] = c2                                 # dist const term
    tbl[SCOL0:SCOL1, K : 2 * K] = WW.T                   # t linear term
    tbl[SCOL1, K : 2 * K] = BB                           # t const term
    ident = np.eye(128, dtype=np.float32)
    wtr = np.broadcast_to(
        np.asarray(W_tune, np.float32).reshape(1, H), (128, H)
    ).copy()
    if HID_BF16:
        import ml_dtypes

        wtr = wtr.astype(ml_dtypes.bfloat16)
    return tbl, ident, wtr, float(np.asarray(b_tune).reshape(-1)[0])


def make_xs1(x, s):
    rows = x.shape[0]
    xs1 = np.empty((rows, NF), np.float32)
    xs1[:, 0:DX] = x
    xs1[:, XCOL1] = 1.0
    xs1[:, SCOL0:SCOL1] = s
    xs1[:, SCOL1] = 1.0
    return xs1


def make_in_maps(inputs, rows_per_core, n_cores=NCORES):
    x = np.asarray(inputs["x"], np.float32)
    s = np.asarray(inputs["s"], np.float32)
    hidden = np.ascontiguousarray(np.asarray(inputs["hidden"], np.float32))
    naive = np.ascontiguousarray(np.asarray(inputs["naive_pred"], np.float32))
    xs1 = make_xs1(x, s)
    tbl, ident, wtr, b_tune_val = _prep_tables(
        inputs["centers"], inputs["W_tune"], inputs["b_tune"],
        inputs["W_sn"], inputs["b_sn"], inputs["running_sn_weight"],
    )
    in_maps = []
    for i in range(n_cores):
        r0 = i * rows_per_core
        in_maps.append(
            {
                "xs1": xs1[r0 : r0 + rows_per_core],
                "hidden": hidden[r0 : r0 + rows_per_core],
                "naive": naive[r0 : r0 + rows_per_core],
                "bigtable": tbl,
                "ident": ident,
                "wtune_rep": wtr,
            }
        )
    return in_maps, b_tune_val


def _run(inputs, rows_per_core=ROWS, n_cores=NCORES, trace=False, tmpdir=None):
    in_maps, b_tune_val = make_in_maps(inputs, rows_per_core, n_cores)
    nc = _build(rows_per_core, b_tune_val)
    bres = run_bass_kernel_spmd(
        nc, in_maps, core_ids=list(range(n_cores)), trace=trace, tmpdir=tmpdir
    )
    out = np.concatenate([r["out"] for r in bres.results], axis=0)
    return out, bres


def kernel(**inputs) -> np.ndarray:
    out, _ = _run(inputs)
    return out
